# revision 1
# baseline (speedup 1.0000x reference)
"""DeStationaryAttention Trainium2 kernel.

Full inputs in, full output out. Sharding: B*N = 64 attention heads are
split across 8 NeuronCores, 8 heads each: core c handles batch b = c//2,
nodes n0 = (c%2)*8 .. n0+8. Inputs are pre-sliced on the host so each
core receives contiguous [T=1024, H=8, D=128] tensors.

Per-head math (T=1024, D=128):
    Qc = Q - mean_T(Q)
    tau = 2*sigmoid(mean_T(std)*w + b)          (scalar per head)
    S[t,s] = Qc[t]·Kc[s] / sqrt(D)
    out = softmax(tau*S) @ V
K-centering is dropped: softmax_s(Qc·(K-muK)) == softmax_s(Qc·K) because
the Qc[t]·muK term is constant along s. Exponent args are bounded (|.| ≲ 10)
so no max-subtraction is needed in fp32.

Device layout per head:
    qcT,kT = [D=128 part, T free] via PE transposes; Q-centering is fused
             into the PSUM evacuation (ScalarE Identity + per-partition bias)
    S^T    = kT_slice.T @ qcT  (fp32r matmuls, N=512 -> full PE rate)
    E^T    = exp(tau_scale * S^T) on ScalarE (PSUM -> SBUF), tau from a
             device-side prologue (exp/recip form of sigmoid, one table set)
    O^T   += V_nat_slice.T @ E^T  (fp32r, accumulated in PSUM)
    rowsum = per-t-tile mini-matmuls over Esum = sum_i E^T_i (DVE add chain)
    out    = PE-transpose(O^T) * (1/rowsum)  -> natural [T,D] -> HBM

Emission is software-pipelined (prep(h+1) before finalize(h), finalize
interleaved one s-tile into sloop(h+1)) so engine queues overlap heads.
"""

import os
import sys
from contextlib import ExitStack

for _p in ("/root/.axon_site/_ro/trn_rl_repo", "/opt/trn_rl_repo"):
    if os.path.isdir(_p) and _p not in sys.path:
        sys.path.append(_p)

import numpy as np

import concourse.bass as bass
import concourse.mybir as mybir
import concourse.tile as tile
from concourse import bacc
from concourse.bass_utils import run_bass_kernel_spmd
from concourse.masks import make_identity

B, T, N, D = 4, 1024, 16, 128
H = 8           # heads per core
NCORES = 8
TT = T // 128   # 128-row tiles along T
F32 = mybir.dt.float32
F32R = mybir.dt.float32r
SCALE2 = 2.0 * D ** (-0.5)   # folded 2*sigmoid(...) * D^-0.5 broadcast constant


def _r(ap):
    return ap.bitcast(F32R)


def _emit(tc):
    nc = tc.nc
    q_d = nc.dram_tensor("Q", [T, H, D], F32, kind="ExternalInput").ap()
    k_d = nc.dram_tensor("K", [T, H, D], F32, kind="ExternalInput").ap()
    v_d = nc.dram_tensor("V", [T, H, D], F32, kind="ExternalInput").ap()
    std_d = nc.dram_tensor("S", [T, H], F32, kind="ExternalInput").ap()
    tw_d = nc.dram_tensor("TW", [1, 1], F32, kind="ExternalInput").ap()
    tb_d = nc.dram_tensor("TB", [1, 1], F32, kind="ExternalInput").ap()
    o_d = nc.dram_tensor("O", [T, H, D], F32, kind="ExternalOutput").ap()

    Exp = mybir.ActivationFunctionType.Exp
    X = mybir.AxisListType.X

    ctx = ExitStack()
    const = ctx.enter_context(tc.tile_pool(name="const", bufs=1))
    nat = ctx.enter_context(tc.tile_pool(name="nat", bufs=3))
    big = ctx.enter_context(tc.tile_pool(name="big", bufs=3))
    etp = ctx.enter_context(tc.tile_pool(name="etp", bufs=6))
    esp = ctx.enter_context(tc.tile_pool(name="esp", bufs=2))
    otsp = ctx.enter_context(tc.tile_pool(name="otsp", bufs=3))
    onatp = ctx.enter_context(tc.tile_pool(name="onatp", bufs=3))
    smallp = ctx.enter_context(tc.tile_pool(name="smallp", bufs=3))
    ps_st = ctx.enter_context(tc.tile_pool(name="ps_st", bufs=2, space="PSUM"))
    ps_ot = ctx.enter_context(tc.tile_pool(name="ps_ot", bufs=1, space="PSUM"))
    ps_sm = ctx.enter_context(tc.tile_pool(name="ps_sm", bufs=2, space="PSUM"))

    # constants
    ident = const.tile([128, 128], F32)
    make_identity(nc, ident)
    ones128 = const.tile([128, 1], F32)
    nc.vector.memset(ones128, 1.0)
    inv_t = const.tile([128, 1], F32)
    nc.vector.memset(inv_t, 1.0 / T)
    bc2 = const.tile([1, 128], F32)
    nc.vector.memset(bc2, SCALE2)

    std_sb = const.tile([128, T * H // 128], F32)   # [128, 64] contiguous
    nc.sync.dma_start(out=std_sb, in_=std_d.rearrange("(p j) h -> p (j h)", p=128))
    tw_sb = const.tile([1, 1], F32)
    nc.sync.dma_start(out=tw_sb, in_=tw_d)
    tb_sb = const.tile([1, 1], F32)
    nc.sync.dma_start(out=tb_sb, in_=tb_d)
    negw = const.tile([1, 1], F32)
    nc.vector.tensor_scalar_mul(negw, tw_sb, -1.0)
    negb = const.tile([1, 1], F32)
    nc.vector.tensor_scalar_mul(negb, tb_sb, -1.0)

    std3 = std_sb.rearrange("p (j h) -> p j h", h=H)
    Ident = mybir.ActivationFunctionType.Identity

    # ---- tau prologue (emitted after prep(0) so transposes overlap it) ----
    taup = ctx.enter_context(tc.tile_pool(name="taup", bufs=H))
    tau_scs = []

    def emit_taus():
      for h in range(H):
          part = smallp.tile([128, 1], F32, tag="part")
          nc.vector.reduce_sum(out=part, in_=std3[:, :, h], axis=X)
          mean_ps = ps_sm.tile([1, 1], F32, tag="ps_sm")
          nc.tensor.matmul(mean_ps, lhsT=inv_t, rhs=part, start=True, stop=True)
          ez = smallp.tile([1, 1], F32, tag="ez")
          nc.scalar.activation(ez, mean_ps, Exp, bias=negb[:], scale=negw[:])
          den = smallp.tile([1, 1], F32, tag="den")
          nc.vector.tensor_scalar_add(den, ez, 1.0)
          sig = smallp.tile([1, 1], F32, tag="sig")
          nc.vector.reciprocal(sig, den)
          tau_ps = ps_sm.tile([128, 1], F32, tag="ps_sm")
          nc.tensor.matmul(tau_ps, lhsT=bc2, rhs=sig, start=True, stop=True)
          tau_sc = taup.tile([128, 1], F32, tag="tau_sc")
          nc.vector.tensor_copy(tau_sc, tau_ps)
          tau_scs.append(tau_sc)

    def prep(h):
        # loads (natural [t_mod, tt, d] tiling) + transposes + fused centering
        q_nat = nat.tile([128, TT, 128], F32, tag="q_nat")
        nc.sync.dma_start(out=q_nat, in_=q_d[:, h, :].rearrange("(tt p) d -> p tt d", p=128))
        k_nat = nat.tile([128, TT, 128], F32, tag="k_nat")
        nc.sync.dma_start(out=k_nat, in_=k_d[:, h, :].rearrange("(tt p) d -> p tt d", p=128))
        v_nat = nat.tile([128, TT, 128], F32R, tag="v_nat")
        nc.sync.dma_start(out=v_nat, in_=_r(v_d[:, h, :].rearrange("(tt p) d -> p tt d", p=128)))

        qcT = big.tile([128, T], F32R, tag="qcT")
        kT = big.tile([128, T], F32R, tag="kT")
        qpacks = []
        mups = []
        for a in range(TT // 4):
            qpack = ps_sm.tile([128, 512], F32, tag="ps_sm")
            for j in range(4):
                nc.tensor.transpose(qpack[:, j * 128:(j + 1) * 128], q_nat[:, a * 4 + j, :], ident)
            qpacks.append(qpack)
            mup = smallp.tile([128, 1], F32, tag="mup%d" % a)
            nc.vector.reduce_sum(out=mup, in_=qpack, axis=X)
            mups.append(mup)
        musum = smallp.tile([128, 1], F32, tag="musum")
        nc.vector.tensor_add(musum, mups[0], mups[1])
        nmu = smallp.tile([128, 1], F32, tag="nmu")
        nc.vector.tensor_scalar_mul(nmu, musum, -1.0 / T)
        for a in range(TT // 4):
            nc.scalar.activation(qcT[:, a * 512:(a + 1) * 512], qpacks[a], Ident,
                                 bias=nmu[:], scale=1.0)
        for a in range(TT // 4):
            kpack = ps_sm.tile([128, 512], F32, tag="ps_sm")
            for j in range(4):
                nc.tensor.transpose(kpack[:, j * 128:(j + 1) * 128], k_nat[:, a * 4 + j, :], ident)
            nc.scalar.activation(kT[:, a * 512:(a + 1) * 512], kpack,
                                 mybir.ActivationFunctionType.Copy)
        return {"qcT": qcT, "kT": kT, "v_nat": v_nat}

    def sloop(h, st, lo=0, hi=TT):
        qcT, kT, v_nat = st["qcT"], st["kT"], st["v_nat"]
        tau_sc = tau_scs[h]
        if lo == 0:
            st["ot_ps"] = ps_ot.tile([128, T], F32, tag="ps_ot", name="ot_ps")
            st["esum"] = esp.tile([128, T], F32, tag="esum", name="esum")
            st["prev_et"] = None
        ot_ps = st["ot_ps"]
        esum = st["esum"]
        prev_et = st["prev_et"]
        def emit_av(i, et):
            vlhs = v_nat[:, i, :]
            nc.tensor.matmul(ot_ps[:, 0:512], lhsT=vlhs, rhs=et[:, 0:512], start=(i == 0), stop=(i == TT - 1))
            nc.tensor.matmul(ot_ps[:, 512:1024], lhsT=vlhs, rhs=et[:, 512:1024], start=(i == 0), stop=(i == TT - 1))

        # in-loop software pipeline: S-matmuls of tile i are emitted before the
        # AV-matmuls of tile i-1, so the PE queue never parks on an AV whose
        # exp hasn't finished while the next S could run.
        pend = st.get("pend_av") or []
        for i in range(lo, hi):
            st_ps = ps_st.tile([128, T], F32, tag="ps_st")
            klhs = kT[:, i * 128:(i + 1) * 128]
            nc.tensor.matmul(st_ps[:, 0:512], lhsT=klhs, rhs=qcT[:, 0:512], start=True, stop=True)
            nc.tensor.matmul(st_ps[:, 512:1024], lhsT=klhs, rhs=qcT[:, 512:1024], start=True, stop=True)
            et = etp.tile([128, T], F32R, tag="et")
            nc.scalar.activation(et, st_ps, Exp, bias=0.0, scale=tau_sc[:])
            pend.append((i, et))
            if len(pend) > 2:
                emit_av(*pend.pop(0))
            if i == 1:
                nc.vector.tensor_add(esum, prev_et.bitcast(F32), et.bitcast(F32))
            elif i > 1:
                nc.vector.tensor_add(esum, esum, et.bitcast(F32))
            prev_et = et
        if hi == TT:
            while pend:
                emit_av(*pend.pop(0))
        st["pend_av"] = pend
        st["prev_et"] = prev_et

    def finalize(h, st):
        esum, ot_ps = st["esum"], st["ot_ps"]
        rs_ps = ps_sm.tile([128, TT], F32, tag="ps_sm")
        for tt in range(TT):
            nc.tensor.matmul(rs_ps[:, tt:tt + 1], lhsT=esum[:, tt * 128:(tt + 1) * 128],
                             rhs=ones128, start=True, stop=True)
        recipT = smallp.tile([128, TT], F32, tag="recipT")
        nc.vector.reciprocal(recipT, rs_ps)

        ots = otsp.tile([128, T], F32, tag="ots")
        nc.scalar.copy(ots, ot_ps)
        o_nat = onatp.tile([128, TT, 128], F32, tag="o_nat")
        for a in range(TT // 4):
            fpack = ps_sm.tile([128, 512], F32, tag="ps_sm")
            for j in range(4):
                tt = a * 4 + j
                nc.tensor.transpose(fpack[:, j * 128:(j + 1) * 128], ots[:, tt * 128:(tt + 1) * 128], ident)
            for j in range(4):
                tt = a * 4 + j
                nc.vector.tensor_scalar_mul(o_nat[:, tt, :], fpack[:, j * 128:(j + 1) * 128],
                                            recipT[:, tt:tt + 1])
        nc.sync.dma_start(out=o_d[:, h, :].rearrange("(tt p) d -> p tt d", p=128), in_=o_nat)

    # software-pipelined emission: head h+1's prep lands on each engine's
    # queue BEFORE head h's finalize, so the inter-head transpose/centering
    # chain overlaps the previous head's tail instead of serializing after it.
    states = [None] * H
    emit_taus()
    states[0] = prep(0)
    sloop(0, states[0])
    for h in range(1, H):
        states[h] = prep(h)
        sloop(h, states[h], 0, 1)
        finalize(h - 1, states[h - 1])
        sloop(h, states[h], 1, TT)
    finalize(H - 1, states[H - 1])
    ctx.close()


_BUILT = None


def _build():
    global _BUILT
    if _BUILT is None:
        nc = bacc.Bacc("TRN2", target_bir_lowering=False, debug=False, num_devices=None)
        with tile.TileContext(nc) as tc:
            _emit(tc)
        nc.compile()
        _BUILT = nc
    return _BUILT


def _in_maps(Q, K, V, std, tau_w, tau_b):
    tw = np.asarray(tau_w, np.float32).reshape(1, 1)
    tb = np.asarray(tau_b, np.float32).reshape(1, 1)
    maps = []
    for c in range(NCORES):
        b, n0 = c // 2, (c % 2) * H
        maps.append({
            "Q": np.ascontiguousarray(Q[b, :, n0:n0 + H, :], np.float32),
            "K": np.ascontiguousarray(K[b, :, n0:n0 + H, :], np.float32),
            "V": np.ascontiguousarray(V[b, :, n0:n0 + H, :], np.float32),
            "S": np.ascontiguousarray(std[b, :, n0:n0 + H, 0], np.float32),
            "TW": tw,
            "TB": tb,
        })
    return maps


def _gather(results):
    out = np.empty((B, T, N, D), np.float32)
    for c in range(NCORES):
        b, n0 = c // 2, (c % 2) * H
        out[b, :, n0:n0 + H, :] = results[c]["O"]
    return out


def run(Q, K, V, std, tau_w, tau_b, **spmd_kwargs):
    nc = _build()
    res = run_bass_kernel_spmd(nc, _in_maps(Q, K, V, std, tau_w, tau_b),
                               core_ids=list(range(NCORES)), **spmd_kwargs)
    return _gather(res.results), res


def kernel(Q, K, V, std, tau_w, tau_b):
    out, _ = run(Q, K, V, std, tau_w, tau_b)
    return out



# revision 8
# speedup vs baseline: 1.0094x; 1.0094x over previous
"""DeStationaryAttention Trainium2 kernel (v2 — transpose-free).

Full inputs in, full output out. Sharding: B*N = 64 attention heads are
split across 8 NeuronCores, 8 heads each: core c handles batch b = c//2,
nodes n0 = (c%2)*8 .. n0+8.

Host-side prep is pure layout (slice / transpose / dtype-pack):
  QK  [H, 2, 128, 1024] f32  — Q^T and K^T per head ([d, t] major)
  V   [H, 128, 8, 128]  bf16 — V tiled [t%128, t//128, d]
  S   [1024, H] f32 (std), TW/TB [1, 1] — tau Linear params
Device returns OT [H, 128, 1024] f32 — the normalized attention output
transposed ([d, t]); host transposes back to [t, d].

Per-head math (T=1024, D=128):
  tau_eff = 2*sigmoid(mean_T(std)*w + b) * D^-0.5       (device prologue)
  qct     = (qT - mean_T(q)) * tau_eff                  (one DVE tensor_scalar)
  per s-tile i (8 of 128 rows):
    S^T_i = kT_i.T @ qct          (PE fp32r, 2 matmuls N=512)
    E^T_i = exp(S^T_i)            (ScalarE, PSUM->SBUF, bf16 out)
    O^T  += V_i.T @ E^T_i         (PE bf16, accumulated in PSUM)
  esum  = pairwise-tree sum of E tiles                  (DVE bf16 2x mode)
  rsbc  = ones128.T @ esum  — row sums broadcast to all partitions (PE)
  out   = O^T * reciprocal(rsbc)                        (DVE, evacuates PSUM)
K-centering is dropped: softmax_s(Qc·(K-muK)) == softmax_s(Qc·K).

Emission is software-pipelined across heads: DMA prefetch 2 heads ahead,
centering for head h+1 and finalize for head h-1 are interleaved into
head h's s-loop, and head h+1's first S/exp is emitted before head h's
tail AV matmuls so ScalarE never idles at head boundaries.
"""

import os
import sys
from contextlib import ExitStack

for _p in ("/root/.axon_site/_ro/trn_rl_repo", "/opt/trn_rl_repo"):
    if os.path.isdir(_p) and _p not in sys.path:
        sys.path.append(_p)

import numpy as np
import ml_dtypes

import concourse.bass as bass
import concourse.mybir as mybir
import concourse.tile as tile
from concourse import bacc
from concourse.bass_utils import run_bass_kernel_spmd

B, T, N, D = 4, 1024, 16, 128
H = 8           # heads per core
NCORES = 8
TT = T // 128   # 128-row tiles along T
F32 = mybir.dt.float32
F32R = mybir.dt.float32r
BF16 = mybir.dt.bfloat16
SCALE2 = 2.0 * D ** (-0.5)   # folds the 2*sigmoid(...) and D^-0.5 scales


def _r(ap):
    return ap.bitcast(F32R)


def _emit(tc):
    nc = tc.nc
    qk_d = nc.dram_tensor("QK", [H, D, 2, T], F32, kind="ExternalInput").ap()
    v_d = nc.dram_tensor("V", [H, 128, TT, D], BF16, kind="ExternalInput").ap()
    std_d = nc.dram_tensor("S", [T, H], F32, kind="ExternalInput").ap()
    tw_d = nc.dram_tensor("TW", [1, 1], F32, kind="ExternalInput").ap()
    tb_d = nc.dram_tensor("TB", [1, 1], F32, kind="ExternalInput").ap()
    o_d = nc.dram_tensor("O", [H, D, T], F32, kind="ExternalOutput").ap()

    Exp = mybir.ActivationFunctionType.Exp
    X = mybir.AxisListType.X
    Add = mybir.AluOpType.add
    Mult = mybir.AluOpType.mult

    ctx = ExitStack()
    const = ctx.enter_context(tc.tile_pool(name="const", bufs=1))
    qkp = ctx.enter_context(tc.tile_pool(name="qkp", bufs=3))
    etp = ctx.enter_context(tc.tile_pool(name="etp", bufs=6))
    trp = ctx.enter_context(tc.tile_pool(name="trp", bufs=3))
    finp = ctx.enter_context(tc.tile_pool(name="finp", bufs=2))
    smallp = ctx.enter_context(tc.tile_pool(name="smallp", bufs=2))
    taup = ctx.enter_context(tc.tile_pool(name="taup", bufs=H))
    ps = ctx.enter_context(tc.tile_pool(name="ps", bufs=2, space="PSUM"))

    # ---- constants ----
    ones_bf = const.tile([128, 128], BF16)
    nc.vector.memset(ones_bf, 1.0)
    inv_t = const.tile([128, 1], F32)
    nc.vector.memset(inv_t, 1.0 / T)
    bc2 = const.tile([1, 128], F32)
    nc.vector.memset(bc2, SCALE2)
    std_sb = const.tile([128, T * H // 128], F32)   # [128, 64]
    nc.sync.dma_start(out=std_sb, in_=std_d.rearrange("(p j) h -> p (j h)", p=128))
    tw_sb = const.tile([1, 1], F32)
    nc.sync.dma_start(out=tw_sb, in_=tw_d)
    tb_sb = const.tile([1, 1], F32)
    nc.sync.dma_start(out=tb_sb, in_=tb_d)
    negw = const.tile([1, 1], F32)
    nc.vector.tensor_scalar_mul(negw, tw_sb, -1.0)
    negb = const.tile([1, 1], F32)
    nc.vector.tensor_scalar_mul(negb, tb_sb, -1.0)
    std3 = std_sb.rearrange("p (j h) -> p j h", h=H)

    # ---- per-head tau_eff = 2*sigmoid(mean(std)*w+b)*D^-0.5, as [128,1] ----
    tau_scs = []
    for h in range(H):
        part = smallp.tile([128, 1], F32, tag="part")
        nc.vector.reduce_sum(out=part, in_=std3[:, :, h], axis=X)
        mean_ps = ps.tile([1, 1], F32, tag="st")
        nc.tensor.matmul(mean_ps, lhsT=inv_t, rhs=part, start=True, stop=True)
        ez = smallp.tile([1, 1], F32, tag="ez")
        nc.scalar.activation(ez, mean_ps, Exp, bias=negb[:], scale=negw[:])
        den = smallp.tile([1, 1], F32, tag="den")
        nc.vector.tensor_scalar_add(den, ez, 1.0)
        sig = smallp.tile([1, 1], F32, tag="sig")
        nc.vector.reciprocal(sig, den)
        tau_ps = ps.tile([128, 1], F32, tag="st")
        nc.tensor.matmul(tau_ps, lhsT=bc2, rhs=sig, start=True, stop=True)
        tau_sc = taup.tile([128, 1], F32, tag="tau_sc")
        nc.vector.tensor_copy(tau_sc, tau_ps)
        tau_scs.append(tau_sc)

    states = [dict() for _ in range(H)]

    def prep_dma(h):
        st = states[h]
        qk = qkp.tile([128, 2 * T], F32R, tag="qk", name="qk")
        nc.sync.dma_start(out=qk, in_=_r(qk_d[h].rearrange("d two t -> d (two t)")))
        v = qkp.tile([128, TT, 128], BF16, tag="v", name="v")
        nc.sync.dma_start(out=v, in_=v_d[h])
        st["qk"], st["v"] = qk, v

    def prep_center(h):
        # DVE: column means of qT, then qct = (qT - mu) * tau_eff
        st = states[h]
        qT = st["qk"][:, 0:T].bitcast(F32)
        qsum = smallp.tile([128, 1], F32, tag="qsum")
        nc.vector.reduce_sum(out=qsum, in_=qT, axis=X)
        negmu = smallp.tile([128, 1], F32, tag="negmu")
        nc.vector.tensor_scalar_mul(negmu, qsum, -1.0 / T)
        qct = qkp.tile([128, T], F32R, tag="qct", name="qct")
        nc.vector.tensor_scalar(qct, qT, negmu[:], tau_scs[h][:], op0=Add, op1=Mult)
        st["qct"] = qct

    def emit_av(st, i, et_slice):
        ot, v = st["ot"], st["v"]
        vl = v[:, i, :]
        nc.tensor.matmul(ot[:, 0:512], lhsT=vl, rhs=et_slice[:, 0:512],
                         start=(i == 0), stop=(i == TT - 1))
        nc.tensor.matmul(ot[:, 512:1024], lhsT=vl, rhs=et_slice[:, 512:1024],
                         start=(i == 0), stop=(i == TT - 1))

    def fin_pe(h):
        # row sums of E, broadcast to all partitions: rsbc = ones128.T @ esum
        st = states[h]
        esum = st["esum"]
        rsbc = ps.tile([128, T], F32, tag="st", name="rsbc")
        nc.tensor.matmul(rsbc[:, 0:512], lhsT=ones_bf, rhs=esum[:, 0:512],
                         start=True, stop=True)
        nc.tensor.matmul(rsbc[:, 512:1024], lhsT=ones_bf, rhs=esum[:, 512:1024],
                         start=True, stop=True)
        st["rsbc"] = rsbc

    def fin_dve(h):
        # out = O^T * (1/rowsum); both ops evacuate PSUM as they go
        st = states[h]
        rcp = finp.tile([128, T], F32, tag="rcp", name="rcp")
        nc.vector.reciprocal(rcp, st["rsbc"])
        ots = finp.tile([128, T], F32, tag="ots", name="ots")
        nc.vector.tensor_mul(ots, st["ot"], rcp)
        nc.sync.dma_start(out=o_d[h], in_=ots)

    def sloop(h, lo, hi):
        st = states[h]
        qct = st["qct"]
        kT = st["qk"][:, T:2 * T]
        if lo == 0:
            st["ot"] = ps.tile([128, T], F32, tag="ot", name="ot")
            st["pairs"] = []
            st["pend"] = []
        pairs, pend = st["pairs"], st["pend"]
        for i in range(lo, hi):
            if i == 1 and h + 2 < H:
                prep_dma(h + 2)
            if i == 2 and h > 0:
                fin_pe(h - 1)
            if i == 3 and h > 0:
                fin_dve(h - 1)
            if i == 4:
                treeA = trp.tile([128, 2 * T], BF16, tag="tree", name="treeA")
                nc.vector.tensor_add(treeA, pairs[0], pairs[1])
                st["treeA"] = treeA
            if i == 5 and h + 1 < H:
                prep_center(h + 1)
            klhs = kT[:, i * 128:(i + 1) * 128]
            stp = ps.tile([128, T], F32, tag="st", name="stp")
            nc.tensor.matmul(stp[:, 0:512], lhsT=klhs, rhs=qct[:, 0:512],
                             start=True, stop=True)
            nc.tensor.matmul(stp[:, 512:1024], lhsT=klhs, rhs=qct[:, 512:1024],
                             start=True, stop=True)
            if i % 2 == 0:
                etpair = etp.tile([128, 2 * T], BF16, tag="et", name="etpair")
                pairs.append(etpair)
            etpair = pairs[-1]
            z = (i % 2) * T
            nc.scalar.activation(etpair[:, z:z + T], stp, Exp)
            pend.append((i, etpair[:, z:z + T]))
            if len(pend) > 2:
                emit_av(st, *pend.pop(0))

    def sloop_tail(h):
        # drain AVs, then the esum tree: B = p2+p3, C = A+B, esum = fold(C)
        st = states[h]
        pairs, pend = st["pairs"], st["pend"]
        while pend:
            emit_av(st, *pend.pop(0))
        treeB = trp.tile([128, 2 * T], BF16, tag="tree", name="treeB")
        nc.vector.tensor_add(treeB, pairs[2], pairs[3])
        treeC = trp.tile([128, 2 * T], BF16, tag="tree", name="treeC")
        nc.vector.tensor_add(treeC, st["treeA"], treeB)
        esum = trp.tile([128, T], BF16, tag="esum", name="esum")
        nc.vector.tensor_add(esum, treeC[:, 0:T], treeC[:, T:2 * T])
        st["esum"] = esum

    # ---- software-pipelined emission ----
    prep_dma(0)
    prep_dma(1)
    prep_center(0)
    sloop(0, 0, TT)
    for h in range(1, H):
        # head h's first S/exp lands before head h-1's tail AVs so the
        # ScalarE exp stream never gaps at the head boundary
        sloop(h, 0, 1)
        sloop_tail(h - 1)
        sloop(h, 1, TT)
    sloop_tail(H - 1)
    fin_pe(H - 1)
    fin_dve(H - 1)
    ctx.close()


_BUILT = None


def _build():
    global _BUILT
    if _BUILT is None:
        nc = bacc.Bacc("TRN2", target_bir_lowering=False, debug=False, num_devices=None)
        with tile.TileContext(nc) as tc:
            _emit(tc)
        nc.compile()
        _BUILT = nc
    return _BUILT


def _in_maps(Q, K, V, std, tau_w, tau_b):
    tw = np.asarray(tau_w, np.float32).reshape(1, 1)
    tb = np.asarray(tau_b, np.float32).reshape(1, 1)
    maps = []
    for c in range(NCORES):
        b, n0 = c // 2, (c % 2) * H
        # [T, H, D] -> [H, D, T] transposed views, packed [H, D, 2, T]
        qk = np.empty((H, D, 2, T), np.float32)
        qk[:, :, 0] = Q[b, :, n0:n0 + H, :].transpose(1, 2, 0)
        qk[:, :, 1] = K[b, :, n0:n0 + H, :].transpose(1, 2, 0)
        # V: [T, H, D] -> [H, t%128, t//128, D] in bf16
        v = np.ascontiguousarray(
            V[b, :, n0:n0 + H, :].reshape(TT, 128, H, D).transpose(2, 1, 0, 3)
        ).astype(ml_dtypes.bfloat16)
        maps.append({
            "QK": qk,
            "V": v,
            "S": np.ascontiguousarray(std[b, :, n0:n0 + H, 0], np.float32),
            "TW": tw,
            "TB": tb,
        })
    return maps


def _gather(results):
    out = np.empty((B, T, N, D), np.float32)
    for c in range(NCORES):
        b, n0 = c // 2, (c % 2) * H
        # OT [H, D, T] -> [T, H, D]
        out[b, :, n0:n0 + H, :] = results[c]["O"].transpose(2, 0, 1)
    return out


def run(Q, K, V, std, tau_w, tau_b, **spmd_kwargs):
    nc = _build()
    res = run_bass_kernel_spmd(nc, _in_maps(Q, K, V, std, tau_w, tau_b),
                               core_ids=list(range(NCORES)), **spmd_kwargs)
    return _gather(res.results), res


def kernel(Q, K, V, std, tau_w, tau_b):
    out, _ = run(Q, K, V, std, tau_w, tau_b)
    return out


# revision 10
# speedup vs baseline: 1.4088x; 1.3956x over previous
"""DeStationaryAttention Trainium2 kernel (v2 — transpose-free).

Full inputs in, full output out. Sharding: B*N = 64 attention heads are
split across 8 NeuronCores, 8 heads each: core c handles batch b = c//2,
nodes n0 = (c%2)*8 .. n0+8.

Host-side prep is pure layout (slice / transpose / dtype-pack):
  QK  [H, 2, 128, 1024] f32  — Q^T and K^T per head ([d, t] major)
  V   [H, 128, 8, 128]  bf16 — V tiled [t%128, t//128, d]
  S   [1024, H] f32 (std), TW/TB [1, 1] — tau Linear params
Device returns OT [H, 128, 1024] f32 — the normalized attention output
transposed ([d, t]); host transposes back to [t, d].

Per-head math (T=1024, D=128):
  tau_eff = 2*sigmoid(mean_T(std)*w + b) * D^-0.5       (device prologue)
  qct     = (qT - mean_T(q)) * tau_eff                  (one DVE tensor_scalar)
  per s-tile i (8 of 128 rows):
    S^T_i = kT_i.T @ qct          (PE fp32r, 2 matmuls N=512)
    E^T_i = exp(S^T_i)            (ScalarE, PSUM->SBUF, bf16 out)
    O^T  += V_i.T @ E^T_i         (PE bf16, accumulated in PSUM)
  esum  = pairwise-tree sum of E tiles                  (DVE bf16 2x mode)
  rsbc  = ones128.T @ esum  — row sums broadcast to all partitions (PE)
  out   = O^T * reciprocal(rsbc)                        (DVE, evacuates PSUM)
K-centering is dropped: softmax_s(Qc·(K-muK)) == softmax_s(Qc·K).

Emission is software-pipelined across heads: DMA prefetch 2 heads ahead,
centering for head h+1 and finalize for head h-1 are interleaved into
head h's s-loop, and head h+1's first S/exp is emitted before head h's
tail AV matmuls so ScalarE never idles at head boundaries.
"""

import os
import sys
from contextlib import ExitStack

for _p in ("/root/.axon_site/_ro/trn_rl_repo", "/opt/trn_rl_repo"):
    if os.path.isdir(_p) and _p not in sys.path:
        sys.path.append(_p)

import numpy as np
import ml_dtypes

import concourse.bass as bass
import concourse.mybir as mybir
import concourse.tile as tile
from concourse import bacc
from concourse.bass_utils import run_bass_kernel_spmd

B, T, N, D = 4, 1024, 16, 128
H = 8           # heads per core
NCORES = 8
TT = T // 128   # 128-row tiles along T
F32 = mybir.dt.float32
F32R = mybir.dt.float32r
BF16 = mybir.dt.bfloat16
SCALE2 = 2.0 * D ** (-0.5)   # folds the 2*sigmoid(...) and D^-0.5 scales


def _r(ap):
    return ap.bitcast(F32R)


def _emit(tc):
    nc = tc.nc
    qk_d = nc.dram_tensor("QK", [H, D, 2, T], F32, kind="ExternalInput").ap()
    v_d = nc.dram_tensor("V", [H, 128, TT, D], BF16, kind="ExternalInput").ap()
    std_d = nc.dram_tensor("S", [T, H], F32, kind="ExternalInput").ap()
    tw_d = nc.dram_tensor("TW", [1, 1], F32, kind="ExternalInput").ap()
    tb_d = nc.dram_tensor("TB", [1, 1], F32, kind="ExternalInput").ap()
    o_d = nc.dram_tensor("O", [H, D, T], F32, kind="ExternalOutput").ap()

    Exp = mybir.ActivationFunctionType.Exp
    X = mybir.AxisListType.X
    Add = mybir.AluOpType.add
    Mult = mybir.AluOpType.mult

    ctx = ExitStack()
    const = ctx.enter_context(tc.tile_pool(name="const", bufs=1))
    qkp = ctx.enter_context(tc.tile_pool(name="qkp", bufs=3))
    etp = ctx.enter_context(tc.tile_pool(name="etp", bufs=6))
    trp = ctx.enter_context(tc.tile_pool(name="trp", bufs=3))
    finp = ctx.enter_context(tc.tile_pool(name="finp", bufs=2))
    smallp = ctx.enter_context(tc.tile_pool(name="smallp", bufs=2))
    taup = ctx.enter_context(tc.tile_pool(name="taup", bufs=H))
    ps = ctx.enter_context(tc.tile_pool(name="ps", bufs=2, space="PSUM"))

    # ---- constants ----
    ones_bf = const.tile([128, 128], BF16)
    nc.vector.memset(ones_bf, 1.0)
    inv_t = const.tile([128, 1], F32)
    nc.vector.memset(inv_t, 1.0 / T)
    bc2 = const.tile([1, 128], F32)
    nc.vector.memset(bc2, SCALE2)
    std_sb = const.tile([128, T * H // 128], F32)   # [128, 64]
    nc.sync.dma_start(out=std_sb, in_=std_d.rearrange("(p j) h -> p (j h)", p=128))
    tw_sb = const.tile([1, 1], F32)
    nc.sync.dma_start(out=tw_sb, in_=tw_d)
    tb_sb = const.tile([1, 1], F32)
    nc.sync.dma_start(out=tb_sb, in_=tb_d)
    negw = const.tile([1, 1], F32)
    nc.vector.tensor_scalar_mul(negw, tw_sb, -1.0)
    negb = const.tile([1, 1], F32)
    nc.vector.tensor_scalar_mul(negb, tb_sb, -1.0)
    std3 = std_sb.rearrange("p (j h) -> p j h", h=H)

    # ---- per-head tau_eff = 2*sigmoid(mean(std)*w+b)*D^-0.5, as [128,1] ----
    tau_scs = []
    for h in range(H):
        part = smallp.tile([128, 1], F32, tag="part")
        nc.vector.reduce_sum(out=part, in_=std3[:, :, h], axis=X)
        mean_ps = ps.tile([1, 1], F32, tag="st")
        nc.tensor.matmul(mean_ps, lhsT=inv_t, rhs=part, start=True, stop=True)
        ez = smallp.tile([1, 1], F32, tag="ez")
        nc.scalar.activation(ez, mean_ps, Exp, bias=negb[:], scale=negw[:])
        den = smallp.tile([1, 1], F32, tag="den")
        nc.vector.tensor_scalar_add(den, ez, 1.0)
        sig = smallp.tile([1, 1], F32, tag="sig")
        nc.vector.reciprocal(sig, den)
        tau_ps = ps.tile([128, 1], F32, tag="st")
        nc.tensor.matmul(tau_ps, lhsT=bc2, rhs=sig, start=True, stop=True)
        tau_sc = taup.tile([128, 1], F32, tag="tau_sc")
        nc.vector.tensor_copy(tau_sc, tau_ps)
        tau_scs.append(tau_sc)

    states = [dict() for _ in range(H)]

    def prep_dma(h):
        st = states[h]
        qk = qkp.tile([128, 2 * T], F32R, tag="qk", name="qk")
        nc.sync.dma_start(out=qk, in_=_r(qk_d[h].rearrange("d two t -> d (two t)")))
        v = qkp.tile([128, TT, 128], BF16, tag="v", name="v")
        nc.sync.dma_start(out=v, in_=v_d[h])
        st["qk"], st["v"] = qk, v

    def prep_center(h):
        # DVE: column means of qT, then qct = (qT - mu) * tau_eff
        st = states[h]
        qT = st["qk"][:, 0:T].bitcast(F32)
        qsum = smallp.tile([128, 1], F32, tag="qsum")
        nc.vector.reduce_sum(out=qsum, in_=qT, axis=X)
        negmu = smallp.tile([128, 1], F32, tag="negmu")
        nc.vector.tensor_scalar_mul(negmu, qsum, -1.0 / T)
        qct = qkp.tile([128, T], F32R, tag="qct", name="qct")
        nc.vector.tensor_scalar(qct, qT, negmu[:], tau_scs[h][:], op0=Add, op1=Mult)
        st["qct"] = qct

    def emit_av(st, i, et_slice):
        ot, v = st["ot"], st["v"]
        vl = v[:, i, :]
        nc.tensor.matmul(ot[:, 0:512], lhsT=vl, rhs=et_slice[:, 0:512],
                         start=(i == 0), stop=(i == TT - 1))
        nc.tensor.matmul(ot[:, 512:1024], lhsT=vl, rhs=et_slice[:, 512:1024],
                         start=(i == 0), stop=(i == TT - 1))

    def fin_pe(h):
        # row sums of E broadcast to all partitions: rsbc = ones128.T @ esum,
        # accumulating the two halves of tree-C so no DVE fold is needed
        st = states[h]
        e2 = st["esum2"]
        rsbc = ps.tile([128, T], F32, tag="st", name="rsbc")
        for half in range(2):
            s, e = (half == 0), (half == 1)
            nc.tensor.matmul(rsbc[:, 0:512], lhsT=ones_bf,
                             rhs=e2[:, half * T:half * T + 512], start=s, stop=e)
            nc.tensor.matmul(rsbc[:, 512:1024], lhsT=ones_bf,
                             rhs=e2[:, half * T + 512:half * T + 1024], start=s, stop=e)
        st["rsbc"] = rsbc

    def fin_dve(h):
        # out = O^T * (1/rowsum); both ops evacuate PSUM as they go
        st = states[h]
        rcp = finp.tile([128, T], F32, tag="rcp", name="rcp")
        nc.vector.reciprocal_approx_fast(rcp, st["rsbc"])
        ots = finp.tile([128, T], F32, tag="ots", name="ots")
        nc.vector.tensor_mul(ots, st["ot"], rcp)
        nc.gpsimd.dma_start(out=o_d[h], in_=ots)

    def sloop(h, lo, hi):
        st = states[h]
        qct = st["qct"]
        kT = st["qk"][:, T:2 * T]
        if lo == 0:
            st["ot"] = ps.tile([128, T], F32, tag="ot", name="ot")
            st["pairs"] = []
            st["pend"] = []
        pairs, pend = st["pairs"], st["pend"]
        for i in range(lo, hi):
            if i == 1 and h + 2 < H:
                prep_dma(h + 2)
            if i == 2 and h > 0:
                fin_pe(h - 1)
            if i == 3 and h > 0:
                fin_dve(h - 1)
            if i == 4:
                treeA = trp.tile([128, 2 * T], BF16, tag="tree", name="treeA")
                nc.vector.tensor_add(treeA, pairs[0], pairs[1])
                st["treeA"] = treeA
            if i == 5 and h + 1 < H:
                prep_center(h + 1)
            klhs = kT[:, i * 128:(i + 1) * 128]
            stp = ps.tile([128, T], F32, tag="st", name="stp")
            nc.tensor.matmul(stp[:, 0:512], lhsT=klhs, rhs=qct[:, 0:512],
                             start=True, stop=True)
            nc.tensor.matmul(stp[:, 512:1024], lhsT=klhs, rhs=qct[:, 512:1024],
                             start=True, stop=True)
            if i % 2 == 0:
                etpair = etp.tile([128, 2 * T], BF16, tag="et", name="etpair")
                pairs.append(etpair)
            etpair = pairs[-1]
            z = (i % 2) * T
            nc.scalar.activation(etpair[:, z:z + T], stp, Exp)
            pend.append((i, etpair[:, z:z + T]))
            if len(pend) > 2:
                emit_av(st, *pend.pop(0))

    def sloop_tail(h):
        # drain AVs, then the esum tree: B = p2+p3, C = A+B, esum = fold(C)
        st = states[h]
        pairs, pend = st["pairs"], st["pend"]
        while pend:
            emit_av(st, *pend.pop(0))
        treeB = trp.tile([128, 2 * T], BF16, tag="tree", name="treeB")
        nc.vector.tensor_add(treeB, pairs[2], pairs[3])
        treeC = trp.tile([128, 2 * T], BF16, tag="tree", name="treeC")
        nc.vector.tensor_add(treeC, st["treeA"], treeB)
        st["esum2"] = treeC

    # ---- software-pipelined emission ----
    prep_dma(0)
    prep_dma(1)
    prep_center(0)
    sloop(0, 0, TT)
    for h in range(1, H):
        # head h's first S/exp lands before head h-1's tail AVs so the
        # ScalarE exp stream never gaps at the head boundary
        sloop(h, 0, 1)
        sloop_tail(h - 1)
        sloop(h, 1, TT)
    sloop_tail(H - 1)
    fin_pe(H - 1)
    fin_dve(H - 1)
    ctx.close()


_BUILT = None


def _build():
    global _BUILT
    if _BUILT is None:
        nc = bacc.Bacc("TRN2", target_bir_lowering=False, debug=False, num_devices=None)
        with tile.TileContext(nc) as tc:
            _emit(tc)
        nc.compile()
        _BUILT = nc
    return _BUILT


def _in_maps(Q, K, V, std, tau_w, tau_b):
    tw = np.asarray(tau_w, np.float32).reshape(1, 1)
    tb = np.asarray(tau_b, np.float32).reshape(1, 1)
    maps = []
    for c in range(NCORES):
        b, n0 = c // 2, (c % 2) * H
        # [T, H, D] -> [H, D, T] transposed views, packed [H, D, 2, T]
        qk = np.empty((H, D, 2, T), np.float32)
        qk[:, :, 0] = Q[b, :, n0:n0 + H, :].transpose(1, 2, 0)
        qk[:, :, 1] = K[b, :, n0:n0 + H, :].transpose(1, 2, 0)
        # V: [T, H, D] -> [H, t%128, t//128, D] in bf16
        v = np.ascontiguousarray(
            V[b, :, n0:n0 + H, :].reshape(TT, 128, H, D).transpose(2, 1, 0, 3)
        ).astype(ml_dtypes.bfloat16)
        maps.append({
            "QK": qk,
            "V": v,
            "S": np.ascontiguousarray(std[b, :, n0:n0 + H, 0], np.float32),
            "TW": tw,
            "TB": tb,
        })
    return maps


def _gather(results):
    out = np.empty((B, T, N, D), np.float32)
    for c in range(NCORES):
        b, n0 = c // 2, (c % 2) * H
        # OT [H, D, T] -> [T, H, D]
        out[b, :, n0:n0 + H, :] = results[c]["O"].transpose(2, 0, 1)
    return out


def run(Q, K, V, std, tau_w, tau_b, **spmd_kwargs):
    nc = _build()
    res = run_bass_kernel_spmd(nc, _in_maps(Q, K, V, std, tau_w, tau_b),
                               core_ids=list(range(NCORES)), **spmd_kwargs)
    return _gather(res.results), res


def kernel(Q, K, V, std, tau_w, tau_b):
    out, _ = run(Q, K, V, std, tau_w, tau_b)
    return out


# revision 11
# speedup vs baseline: 1.4097x; 1.0006x over previous
"""DeStationaryAttention Trainium2 kernel (v2 — transpose-free).

Full inputs in, full output out. Sharding: B*N = 64 attention heads are
split across 8 NeuronCores, 8 heads each: core c handles batch b = c//2,
nodes n0 = (c%2)*8 .. n0+8.

Host-side prep is pure layout (slice / transpose / dtype-pack):
  QK  [H, 2, 128, 1024] f32  — Q^T and K^T per head ([d, t] major)
  V   [H, 128, 8, 128]  bf16 — V tiled [t%128, t//128, d]
  S   [1024, H] f32 (std), TW/TB [1, 1] — tau Linear params
Device returns OT [H, 128, 1024] f32 — the normalized attention output
transposed ([d, t]); host transposes back to [t, d].

Per-head math (T=1024, D=128):
  tau_eff = 2*sigmoid(mean_T(std)*w + b) * D^-0.5       (device prologue)
  qct     = (qT - mean_T(q)) * tau_eff                  (one DVE tensor_scalar)
  per s-tile i (8 of 128 rows):
    S^T_i = kT_i.T @ qct          (PE fp32r, 2 matmuls N=512)
    E^T_i = exp(S^T_i)            (ScalarE, PSUM->SBUF, bf16 out)
    O^T  += V_i.T @ E^T_i         (PE bf16, accumulated in PSUM)
  esum  = pairwise-tree sum of E tiles                  (DVE bf16 2x mode)
  rsbc  = ones128.T @ esum  — row sums broadcast to all partitions (PE)
  out   = O^T * reciprocal(rsbc)                        (DVE, evacuates PSUM)
K-centering is dropped: softmax_s(Qc·(K-muK)) == softmax_s(Qc·K).

Emission is software-pipelined across heads: DMA prefetch 2 heads ahead,
centering for head h+1 and finalize for head h-1 are interleaved into
head h's s-loop, and head h+1's first S/exp is emitted before head h's
tail AV matmuls so ScalarE never idles at head boundaries.
"""

import os
import sys
from contextlib import ExitStack

for _p in ("/root/.axon_site/_ro/trn_rl_repo", "/opt/trn_rl_repo"):
    if os.path.isdir(_p) and _p not in sys.path:
        sys.path.append(_p)

import numpy as np
import ml_dtypes

import concourse.bass as bass
import concourse.mybir as mybir
import concourse.tile as tile
from concourse import bacc
from concourse.bass_utils import run_bass_kernel_spmd

B, T, N, D = 4, 1024, 16, 128
H = 8           # heads per core
NCORES = 8
TT = T // 128   # 128-row tiles along T
F32 = mybir.dt.float32
F32R = mybir.dt.float32r
BF16 = mybir.dt.bfloat16
SCALE2 = 2.0 * D ** (-0.5)   # folds the 2*sigmoid(...) and D^-0.5 scales


def _r(ap):
    return ap.bitcast(F32R)


def _emit(tc):
    nc = tc.nc
    qk_d = nc.dram_tensor("QK", [H, D, 2, T], F32, kind="ExternalInput").ap()
    v_d = nc.dram_tensor("V", [H, 128, TT, D], BF16, kind="ExternalInput").ap()
    std_d = nc.dram_tensor("S", [T, H], F32, kind="ExternalInput").ap()
    tw_d = nc.dram_tensor("TW", [1, 1], F32, kind="ExternalInput").ap()
    tb_d = nc.dram_tensor("TB", [1, 1], F32, kind="ExternalInput").ap()
    o_d = nc.dram_tensor("O", [H, D, T], F32, kind="ExternalOutput").ap()

    Exp = mybir.ActivationFunctionType.Exp
    X = mybir.AxisListType.X
    Add = mybir.AluOpType.add
    Mult = mybir.AluOpType.mult

    ctx = ExitStack()
    const = ctx.enter_context(tc.tile_pool(name="const", bufs=1))
    qkp = ctx.enter_context(tc.tile_pool(name="qkp", bufs=3))
    etp = ctx.enter_context(tc.tile_pool(name="etp", bufs=6))
    trp = ctx.enter_context(tc.tile_pool(name="trp", bufs=3))
    finp = ctx.enter_context(tc.tile_pool(name="finp", bufs=2))
    smallp = ctx.enter_context(tc.tile_pool(name="smallp", bufs=2))
    taup = ctx.enter_context(tc.tile_pool(name="taup", bufs=H))
    ps = ctx.enter_context(tc.tile_pool(name="ps", bufs=2, space="PSUM"))

    # ---- constants ----
    ones_bf = const.tile([128, 128], BF16)
    nc.vector.memset(ones_bf, 1.0)
    inv_t = const.tile([128, 1], F32)
    nc.vector.memset(inv_t, 1.0 / T)
    bc2 = const.tile([1, 128], F32)
    nc.vector.memset(bc2, SCALE2)
    std_sb = const.tile([128, T * H // 128], F32)   # [128, 64]
    nc.sync.dma_start(out=std_sb, in_=std_d.rearrange("(p j) h -> p (j h)", p=128))
    tw_sb = const.tile([1, 1], F32)
    nc.sync.dma_start(out=tw_sb, in_=tw_d)
    tb_sb = const.tile([1, 1], F32)
    nc.sync.dma_start(out=tb_sb, in_=tb_d)
    negw = const.tile([1, 1], F32)
    nc.vector.tensor_scalar_mul(negw, tw_sb, -1.0)
    negb = const.tile([1, 1], F32)
    nc.vector.tensor_scalar_mul(negb, tb_sb, -1.0)
    std3 = std_sb.rearrange("p (j h) -> p j h", h=H)

    # ---- per-head tau_eff = 2*sigmoid(mean(std)*w+b)*D^-0.5, as [128,1] ----
    tau_scs = []
    for h in range(H):
        part = smallp.tile([128, 1], F32, tag="part")
        nc.vector.reduce_sum(out=part, in_=std3[:, :, h], axis=X)
        mean_ps = ps.tile([1, 1], F32, tag="st", bufs=3)
        nc.tensor.matmul(mean_ps, lhsT=inv_t, rhs=part, start=True, stop=True)
        ez = smallp.tile([1, 1], F32, tag="ez")
        nc.scalar.activation(ez, mean_ps, Exp, bias=negb[:], scale=negw[:])
        den = smallp.tile([1, 1], F32, tag="den")
        nc.vector.tensor_scalar_add(den, ez, 1.0)
        sig = smallp.tile([1, 1], F32, tag="sig")
        nc.vector.reciprocal(sig, den)
        tau_ps = ps.tile([128, 1], F32, tag="st", bufs=3)
        nc.tensor.matmul(tau_ps, lhsT=bc2, rhs=sig, start=True, stop=True)
        tau_sc = taup.tile([128, 1], F32, tag="tau_sc")
        nc.vector.tensor_copy(tau_sc, tau_ps)
        tau_scs.append(tau_sc)

    states = [dict() for _ in range(H)]

    def prep_dma(h):
        st = states[h]
        qk = qkp.tile([128, 2 * T], F32R, tag="qk", name="qk")
        nc.sync.dma_start(out=qk, in_=_r(qk_d[h].rearrange("d two t -> d (two t)")))
        v = qkp.tile([128, TT, 128], BF16, tag="v", name="v")
        nc.sync.dma_start(out=v, in_=v_d[h])
        st["qk"], st["v"] = qk, v

    def prep_center(h):
        # DVE: column means of qT, then qct = (qT - mu) * tau_eff
        st = states[h]
        qT = st["qk"][:, 0:T].bitcast(F32)
        qsum = smallp.tile([128, 1], F32, tag="qsum")
        nc.vector.reduce_sum(out=qsum, in_=qT, axis=X)
        negmu = smallp.tile([128, 1], F32, tag="negmu")
        nc.vector.tensor_scalar_mul(negmu, qsum, -1.0 / T)
        qct = qkp.tile([128, T], F32R, tag="qct", name="qct")
        nc.vector.tensor_scalar(qct, qT, negmu[:], tau_scs[h][:], op0=Add, op1=Mult)
        st["qct"] = qct

    def emit_av(st, i, et_slice):
        ot, v = st["ot"], st["v"]
        vl = v[:, i, :]
        nc.tensor.matmul(ot[:, 0:512], lhsT=vl, rhs=et_slice[:, 0:512],
                         start=(i == 0), stop=(i == TT - 1))
        nc.tensor.matmul(ot[:, 512:1024], lhsT=vl, rhs=et_slice[:, 512:1024],
                         start=(i == 0), stop=(i == TT - 1))

    def fin_pe(h):
        # row sums of E broadcast to all partitions: rsbc = ones128.T @ esum,
        # accumulating the two halves of tree-C so no DVE fold is needed
        st = states[h]
        e2 = st["esum2"]
        rsbc = ps.tile([128, T], F32, tag="st", name="rsbc", bufs=3)
        for half in range(2):
            s, e = (half == 0), (half == 1)
            nc.tensor.matmul(rsbc[:, 0:512], lhsT=ones_bf,
                             rhs=e2[:, half * T:half * T + 512], start=s, stop=e)
            nc.tensor.matmul(rsbc[:, 512:1024], lhsT=ones_bf,
                             rhs=e2[:, half * T + 512:half * T + 1024], start=s, stop=e)
        st["rsbc"] = rsbc

    def fin_dve(h):
        # out = O^T * (1/rowsum); both ops evacuate PSUM as they go
        st = states[h]
        rcp = finp.tile([128, T], F32, tag="rcp", name="rcp")
        nc.vector.reciprocal_approx_fast(rcp, st["rsbc"])
        ots = finp.tile([128, T], F32, tag="ots", name="ots")
        nc.vector.tensor_mul(ots, st["ot"], rcp)
        nc.gpsimd.dma_start(out=o_d[h], in_=ots)

    def sloop(h, lo, hi):
        st = states[h]
        qct = st["qct"]
        kT = st["qk"][:, T:2 * T]
        if lo == 0:
            st["ot"] = ps.tile([128, T], F32, tag="ot", name="ot", bufs=1)
            st["pairs"] = []
            st["pend"] = []
        pairs, pend = st["pairs"], st["pend"]
        for i in range(lo, hi):
            if i == 1 and h + 2 < H:
                prep_dma(h + 2)
            if i == 1 and h > 0:
                fin_pe(h - 1)
            if i == 2 and h > 0:
                fin_dve(h - 1)
            if i == 4:
                treeA = trp.tile([128, 2 * T], BF16, tag="tree", name="treeA")
                nc.vector.tensor_add(treeA, pairs[0], pairs[1])
                st["treeA"] = treeA
            if i == 5 and h + 1 < H:
                prep_center(h + 1)
            klhs = kT[:, i * 128:(i + 1) * 128]
            stp = ps.tile([128, T], F32, tag="st", name="stp", bufs=3)
            nc.tensor.matmul(stp[:, 0:512], lhsT=klhs, rhs=qct[:, 0:512],
                             start=True, stop=True)
            nc.tensor.matmul(stp[:, 512:1024], lhsT=klhs, rhs=qct[:, 512:1024],
                             start=True, stop=True)
            if i % 2 == 0:
                etpair = etp.tile([128, 2 * T], BF16, tag="et", name="etpair")
                pairs.append(etpair)
            etpair = pairs[-1]
            z = (i % 2) * T
            nc.scalar.activation(etpair[:, z:z + T], stp, Exp)
            pend.append((i, etpair[:, z:z + T]))
            if len(pend) > 3:
                emit_av(st, *pend.pop(0))

    def sloop_tail(h):
        # drain AVs, then the esum tree: B = p2+p3, C = A+B, esum = fold(C)
        st = states[h]
        pairs, pend = st["pairs"], st["pend"]
        while pend:
            emit_av(st, *pend.pop(0))
        treeB = trp.tile([128, 2 * T], BF16, tag="tree", name="treeB")
        nc.vector.tensor_add(treeB, pairs[2], pairs[3])
        treeC = trp.tile([128, 2 * T], BF16, tag="tree", name="treeC")
        nc.vector.tensor_add(treeC, st["treeA"], treeB)
        st["esum2"] = treeC

    # ---- software-pipelined emission ----
    prep_dma(0)
    prep_dma(1)
    prep_center(0)
    sloop(0, 0, TT)
    for h in range(1, H):
        # head h's first S/exp lands before head h-1's tail AVs so the
        # ScalarE exp stream never gaps at the head boundary
        sloop(h, 0, 1)
        sloop_tail(h - 1)
        sloop(h, 1, TT)
    sloop_tail(H - 1)
    fin_pe(H - 1)
    fin_dve(H - 1)
    ctx.close()


_BUILT = None


def _build():
    global _BUILT
    if _BUILT is None:
        nc = bacc.Bacc("TRN2", target_bir_lowering=False, debug=False, num_devices=None)
        with tile.TileContext(nc) as tc:
            _emit(tc)
        nc.compile()
        _BUILT = nc
    return _BUILT


def _in_maps(Q, K, V, std, tau_w, tau_b):
    tw = np.asarray(tau_w, np.float32).reshape(1, 1)
    tb = np.asarray(tau_b, np.float32).reshape(1, 1)
    maps = []
    for c in range(NCORES):
        b, n0 = c // 2, (c % 2) * H
        # [T, H, D] -> [H, D, T] transposed views, packed [H, D, 2, T]
        qk = np.empty((H, D, 2, T), np.float32)
        qk[:, :, 0] = Q[b, :, n0:n0 + H, :].transpose(1, 2, 0)
        qk[:, :, 1] = K[b, :, n0:n0 + H, :].transpose(1, 2, 0)
        # V: [T, H, D] -> [H, t%128, t//128, D] in bf16
        v = np.ascontiguousarray(
            V[b, :, n0:n0 + H, :].reshape(TT, 128, H, D).transpose(2, 1, 0, 3)
        ).astype(ml_dtypes.bfloat16)
        maps.append({
            "QK": qk,
            "V": v,
            "S": np.ascontiguousarray(std[b, :, n0:n0 + H, 0], np.float32),
            "TW": tw,
            "TB": tb,
        })
    return maps


def _gather(results):
    out = np.empty((B, T, N, D), np.float32)
    for c in range(NCORES):
        b, n0 = c // 2, (c % 2) * H
        # OT [H, D, T] -> [T, H, D]
        out[b, :, n0:n0 + H, :] = results[c]["O"].transpose(2, 0, 1)
    return out


def run(Q, K, V, std, tau_w, tau_b, **spmd_kwargs):
    nc = _build()
    res = run_bass_kernel_spmd(nc, _in_maps(Q, K, V, std, tau_w, tau_b),
                               core_ids=list(range(NCORES)), **spmd_kwargs)
    return _gather(res.results), res


def kernel(Q, K, V, std, tau_w, tau_b):
    out, _ = run(Q, K, V, std, tau_w, tau_b)
    return out
